# revision 21
# baseline (speedup 1.0000x reference)
"""LoraLinear (int8-dequant matmul + low-rank LoRA) on 8 trn2 NeuronCores.

out[b,s,o] = sum_i x[b,s,i]*q[o,i]*scale[o] + 2.0 * sum_r (sum_i x[b,s,i]*A[r,i]) * B[o,r]

Strategy: data-parallel over the 8192 flattened tokens (1024/core, no
collectives). Host folds scale into the weight, casts operands to bf16
(int8 codes are exact in bf16), and pre-transposes so every DMA is
contiguous. On device each core does a plain bf16 matmul with fp32 PSUM
accumulation; the LoRA term is folded into the same PSUM accumulation
group as one extra K=64 matmul per output tile.
"""

import numpy as np
import ml_dtypes

BF16 = ml_dtypes.bfloat16

B, S, DIN, DOUT, R = 4, 2048, 4096, 4096, 64
N_CORES = 8
TOK = B * S  # 8192
T = TOK // N_CORES  # 1024 tokens per core
P = 128
IC = DIN // P  # 32 contraction chunks
O_TILE = 512
N_OT = DOUT // O_TILE  # 8
N_TT = T // P  # 8
SCALING = 2.0

_CACHE = {}


def build_nc():
    import concourse.mybir as mybir
    import concourse.tile as tile
    from concourse import bacc

    dt = mybir.dt
    nc = bacc.Bacc("TRN2", target_bir_lowering=False, debug=False,
                   num_devices=N_CORES)

    xT_d = nc.dram_tensor("xT", [P, IC, T], dt.bfloat16, kind="ExternalInput").ap()
    wT_d = nc.dram_tensor("wT", [N_OT, P, IC, O_TILE], dt.bfloat16, kind="ExternalInput").ap()
    aT_d = nc.dram_tensor("aT", [P, IC, R], dt.bfloat16, kind="ExternalInput").ap()
    b2T_d = nc.dram_tensor("b2T", [R, DOUT], dt.bfloat16, kind="ExternalInput").ap()
    out_d = nc.dram_tensor("out", [N_OT, N_TT, P, O_TILE], dt.float32, kind="ExternalOutput").ap()

    XCH = 1   # ic per xT tile chunk -> 32 chunks
    WCH = 4   # ic per w tile chunk  -> 8 chunks

    with tile.TileContext(nc) as tc:
        with (
            tc.tile_pool(name="xpool", bufs=1) as xpool,
            tc.tile_pool(name="wpool", bufs=2) as wpool,
            tc.tile_pool(name="cpool", bufs=1) as cpool,
            tc.tile_pool(name="opool", bufs=4) as opool,
            tc.tile_pool(name="psmain", bufs=6, space="PSUM") as psmain,
            tc.tile_pool(name="psxa", bufs=2, space="PSUM") as psxa,
        ):
            # xT and aT split into independently-DMA'd tiles so PE can stream
            # behind the loads (Tile deps are tile-granular).
            ACH = 8
            ats = [cpool.tile([P, ACH, R], dt.bfloat16, tag=f"at{i}", name=f"at{i}")
                   for i in range(IC // ACH)]
            xts = [xpool.tile([P, XCH, T], dt.bfloat16, tag=f"xt{i}", name=f"xt{i}")
                   for i in range(IC // XCH)]

            def x_sl(ic, lo, hi):
                return xts[ic // XCH][:, ic % XCH, lo:hi]

            def a_sl(ic):
                return ats[ic // ACH][:, ic % ACH, :]

            def w_tiles(ot):
                ws = [wpool.tile([P, WCH, O_TILE], dt.bfloat16, tag=f"w{q}", name=f"w_{q}")
                      for q in range(IC // WCH)]
                for q, w in enumerate(ws):
                    nc.sync.dma_start(w[:], wT_d[ot, :, WCH * q:WCH * (q + 1), :])
                return ws

            def w_sl(ws, ic):
                return ws[ic // WCH][:, ic % WCH, :]

            # interleaved DMA emission: x chunks and first w chunks stream
            # together so the ic-outer phase below is PE-bound from the start
            b2T = cpool.tile([R, DOUT], dt.bfloat16)
            w0 = [wpool.tile([P, WCH, O_TILE], dt.bfloat16, tag=f"w{q}", name=f"w0_{q}")
                  for q in range(IC // WCH)]
            nxt = len(xts)
            done_w = 0
            nc.sync.dma_start(ats[0][:], aT_d[:, 0:ACH, :])
            for j in range(nxt):
                nc.sync.dma_start(xts[j][:], xT_d[:, XCH * j:XCH * (j + 1), :])
                if j == 4:
                    nc.sync.dma_start(ats[1][:], aT_d[:, ACH:2 * ACH, :])
                elif j == 10:
                    nc.sync.dma_start(ats[2][:], aT_d[:, 2 * ACH:3 * ACH, :])
                elif j == 16:
                    nc.sync.dma_start(ats[3][:], aT_d[:, 3 * ACH:4 * ACH, :])
                if j % 4 == 1 and done_w < IC // WCH:  # w chunk after every 4th x chunk
                    nc.sync.dma_start(w0[done_w][:],
                                      wT_d[0, :, WCH * done_w:WCH * (done_w + 1), :])
                    done_w += 1
            nc.sync.dma_start(b2T[:], b2T_d[:])
            ws1 = w_tiles(1)  # prefetch ot=1 weights behind the initial load

            xaT = cpool.tile([R, T], dt.bfloat16)
            NB = T // O_TILE  # xa psum blocks (2)

            def lora_and_evict(ps, ot, tt):
                nc.tensor.matmul(
                    ps[:], xaT[:, tt * P:(tt + 1) * P],
                    b2T[:, ot * O_TILE:(ot + 1) * O_TILE],
                    start=False, stop=True,
                )
                st = opool.tile([P, O_TILE], dt.float32)
                # split the eviction across DVE and ACT, each half pipelined
                # straight into its own store DMA, so the post-matmul chain is
                # max(copy)+half-DMA instead of copy+full-DMA
                h = O_TILE // 2
                nc.vector.tensor_copy(out=st[:, :h], in_=ps[:, :h])
                nc.sync.dma_start(out_d[ot, tt, :, 0:h], st[:, :h])
                nc.scalar.copy(st[:, h:], ps[:, h:])
                nc.sync.dma_start(out_d[ot, tt, :, h:O_TILE], st[:, h:])

            # ---- phase 1 (ot=0): ic-outer, xa + 4 token groups interleaved
            ps_xa = [psxa.tile([R, O_TILE], dt.float32, tag="psxa", name=f"psxa{b}") for b in range(NB)]
            NPG = 6
            ps_g = [psmain.tile([P, O_TILE], dt.float32, tag="ps", name=f"psg{g}") for g in range(NPG)]
            for ic in range(IC):
                for tb in range(NB):
                    nc.tensor.matmul(
                        ps_xa[tb][:], a_sl(ic),
                        x_sl(ic, tb * O_TILE, (tb + 1) * O_TILE),
                        start=(ic == 0), stop=(ic == IC - 1),
                    )
                for tt in range(NPG):
                    nc.tensor.matmul(
                        ps_g[tt][:], x_sl(ic, tt * P, (tt + 1) * P), w_sl(w0, ic),
                        start=(ic == 0), stop=False,
                    )
            for tb in range(NB):
                nc.any.tensor_copy(out=xaT[:, tb * O_TILE:(tb + 1) * O_TILE],
                                   in_=ps_xa[tb][:])
            for tt in range(NPG):
                lora_and_evict(ps_g[tt], 0, tt)
            # ot=0 remaining token groups (everything resident)
            for tt in range(NPG, N_TT):
                ps = psmain.tile([P, O_TILE], dt.float32, tag="ps", name="ps")
                for ic in range(IC):
                    nc.tensor.matmul(
                        ps[:], x_sl(ic, tt * P, (tt + 1) * P), w_sl(w0, ic),
                        start=(ic == 0), stop=False,
                    )
                lora_and_evict(ps, 0, tt)

            # ---- steady state: ot = 1..7
            for ot in range(1, N_OT):
                ws = ws1 if ot == 1 else w_tiles(ot)
                for tt in range(N_TT):
                    ps = psmain.tile([P, O_TILE], dt.float32, tag="ps", name="ps")
                    for ic in range(IC):
                        nc.tensor.matmul(
                            ps[:], x_sl(ic, tt * P, (tt + 1) * P), w_sl(ws, ic),
                            start=(ic == 0), stop=False,
                        )
                    lora_and_evict(ps, ot, tt)

    nc.compile()
    return nc


def _prep_inputs(x, qweight, scale, lora_A, lora_B):
    x_flat = np.ascontiguousarray(x.reshape(TOK, DIN))
    # xT per core: [P, IC, T], row i = ic*P + p
    xT_all = x_flat.T.astype(BF16)  # [DIN, TOK]
    per_core_xT = []
    for c in range(N_CORES):
        xs = xT_all[:, c * T:(c + 1) * T]
        per_core_xT.append(np.ascontiguousarray(
            xs.reshape(IC, P, T).transpose(1, 0, 2)))
    # weight with scale folded, transposed: wT[i, o]
    w = qweight.astype(np.float32) * scale.astype(np.float32)  # [DOUT, DIN]
    wT = w.T.astype(BF16)  # [DIN, DOUT]
    wT_t = np.ascontiguousarray(
        wT.reshape(IC, P, N_OT, O_TILE).transpose(2, 1, 0, 3))  # [N_OT, P, IC, O_TILE]
    aT = np.ascontiguousarray(
        lora_A.T.astype(BF16).reshape(IC, P, R).transpose(1, 0, 2))  # [P, IC, R]
    b2T = np.ascontiguousarray((SCALING * lora_B).T.astype(BF16))  # [R, DOUT]
    return per_core_xT, wT_t, aT, b2T


def run(x, qweight, scale, lora_A, lora_B, trace=False):
    from concourse.bass_utils import run_bass_kernel_spmd

    if "nc" not in _CACHE:
        _CACHE["nc"] = build_nc()
    nc = _CACHE["nc"]

    per_core_xT, wT_t, aT, b2T = _prep_inputs(x, qweight, scale, lora_A, lora_B)
    in_maps = [
        {"xT": per_core_xT[c], "wT": wT_t, "aT": aT, "b2T": b2T}
        for c in range(N_CORES)
    ]
    res = run_bass_kernel_spmd(nc, in_maps, core_ids=list(range(N_CORES)),
                               trace=trace)
    outs = []
    for c in range(N_CORES):
        o = res.results[c]["out"]  # [N_OT, N_TT, P, O_TILE]
        outs.append(o.transpose(1, 2, 0, 3).reshape(T, DOUT))
    full = np.concatenate(outs, axis=0).reshape(B, S, DOUT).astype(np.float32)
    return full, res


def kernel(x, qweight, scale, lora_A, lora_B):
    full, _ = run(x, qweight, scale, lora_A, lora_B)
    return full
